# revision 1
# baseline (speedup 1.0000x reference)
"""Trainium2 Bass kernel: batched dot-product attention.

Problem: B=8, N=M=4096, D=64, fp32.
  out[b] = softmax(Q[b] @ K[b].T / sqrt(D)) @ V[b]

Sharding: batch b -> core b (8 cores, no communication).

Per-core algorithm (flash-attention, S^T layout, ACT/exp-throughput bound
-- 16.7M exps on the scalar engine ~ 110 us is the hard floor):
  - Prologue (graduated pieces, overlapped with compute): early Q/K pieces
    are transposed on the PE (fp32 transpose, bf16 cast in the PSUM->SBUF
    copy); the large tail pieces bounce through a bf16 DRAM staging
    [rows,128] and one hardware DMA-transpose each into SBUF Q^T/K^T
    [64,4096] (staging cols 64..127 never written; the transposed garbage
    lands in SBUF partitions 64..127 which nothing reads). V: strided
    load into [128, chunk, 65] with ones in column 64 -> V'.
  - Main loop: OUTER over m-chunk groups (3 chunks of 128 keys per exp
    instruction), INNER over 8 query blocks (NB=512). Two 3-bank S^T PSUM
    pools alternate by inner-iteration parity; with the 2-bank PV-partial
    pool that is exactly 8 PSUM banks. K/V chunks of group g are only
    needed at ~g*15us, so the prologue never stalls compute.
      S^T[mchunk, nb] = (K^T_c).T @ Q^T_blk   (PE, bf16 -> PSUM fp32)
      P^T             = exp(scale * S^T)      (ACT, PSUM -> SBUF bf16)
      partial[nt,65]  = (P^T tile).T @ V'_c   (PE, 1 PSUM bank;
                                               col 64 = softmax denom)
      o_acc[j]       += partial               (DVE, SBUF f32 accumulator)
  - Last group, per block (streams with the inner loop): reciprocal of
    the sums column, one broadcast tensor_tensor multiply (0-stride AP),
    one strided DMA store on the SP HWDGE queue.
"""

import sys

import numpy as np

if "/opt/trn_rl_repo" not in sys.path:
    sys.path.insert(0, "/opt/trn_rl_repo")

import concourse.bass as bass
import concourse.tile as tile
from concourse import bacc, mybir
from concourse.tile import add_dep_helper
from concourse.bass_utils import run_bass_kernel_spmd
from concourse.masks import make_identity

B = 8
SEQ = 4096
D = 64
P = 128

F32 = mybir.dt.float32
BF16 = mybir.dt.bfloat16


def _group_sizes(n_mchunks):
    """Uniform groups of 3 m-chunks (last may be smaller): 32 -> [3]*10+[2].
    The S^T PSUM pools (two, alternating per inner iteration) are 3 banks
    each, leaving 2 banks for the PV partial pool: 3+3+2 = 8."""
    out, c = [], n_mchunks
    while c > 0:
        out.append(min(3, c))
        c -= out[-1]
    return out


def build_nc(seq=SEQ, nb=512, iters=1):
    n_mchunks = seq // P
    n_blocks = seq // nb
    ntiles_blk = nb // P
    scale = 1.0 / np.sqrt(np.float32(D))
    gsizes = _group_sizes(n_mchunks)

    nc = bacc.Bacc("TRN2", target_bir_lowering=False, debug=False)

    q_dram = nc.dram_tensor("queries", [seq, D], F32, kind="ExternalInput")
    k_dram = nc.dram_tensor("keys", [seq, D], F32, kind="ExternalInput")
    v_dram = nc.dram_tensor("values", [seq, D], F32, kind="ExternalInput")
    o_dram = nc.dram_tensor("out", [seq, D], F32, kind="ExternalOutput")

    v_tiled = v_dram.ap().rearrange("(t p) d -> p t d", p=P)
    o_tiled = o_dram.ap().rearrange("(t p) d -> p t d", p=P)

    # graduated prologue pieces (rows). Q is entirely PE-transposed
    # (contiguous loads; late pieces are emitted woven between the first
    # main-loop iterations so the part-pool rotation never blocks PV).
    # K's tail goes through the DRAM staging + DMA-transpose path.
    if seq >= 4096:
        q_pe = [(0, 512), (512, 512), (1024, 1024)]
        q_weave = {0: (2048, 1024), 1: (3072, 1024)}
        k_pe = [(0, 512)]
        k_dma = [(512, 1024), (1536, seq - 1536)]
        v_pieces = [(0, 512), (512, 1024), (1536, seq - 1536)]
    else:
        q_pe = [(0, seq)]
        q_weave = {}
        k_pe = [(0, seq)]
        k_dma = []
        v_pieces = [(0, seq)]

    with tile.TileContext(nc) as tc:
        with (
            tc.tile_pool(name="persist", bufs=1) as persist,
            tc.tile_pool(name="stage", bufs=2) as stage,
            tc.tile_pool(name="dstage", bufs=1, space="DRAM") as dstage,
            tc.tile_pool(name="pexp", bufs=3) as pexp,
            tc.tile_pool(name="outp", bufs=4) as outp,
            tc.tile_pool(name="small", bufs=4) as small,
            tc.tile_pool(name="sga", bufs=1, space=bass.MemorySpace.PSUM) as sgpa,
            tc.tile_pool(name="sgb", bufs=1, space=bass.MemorySpace.PSUM) as sgpb,
            tc.tile_pool(name="part", bufs=2, space=bass.MemorySpace.PSUM) as partp,
        ):
            qt_all = persist.tile([P, seq], BF16, tag="qt")
            kt_all = persist.tile([P, seq], BF16, tag="kt")
            v2 = persist.tile([P, n_mchunks, D + 1], BF16, tag="v2")
            oaccs = [
                persist.tile([P, ntiles_blk, D + 1], F32, tag=f"oa{j}", name=f"oa{j}")
                for j in range(n_blocks)
            ]
            qsd = dstage.tile([seq, P], BF16, tag="qsd")
            ksd = dstage.tile([seq, P], BF16, tag="ksd")
            id_f32 = persist.tile([P, P], F32, tag="idf32")
            make_identity(nc, id_f32)

            # ---------------- prologue (graduated pieces) ----------------
            def qk_piece(name, src, sd, dst, row0, nrows, after=None):
                rows = slice(row0, row0 + nrows)
                rpp = nrows // P
                src_t = src.ap()[rows, :].rearrange("(p r) d -> p r d", r=rpp)
                st_f = stage.tile([P, rpp, D], F32, tag=f"{name}sf", name=f"{name}sf", bufs=4 if name == "q" else 3)
                ld = nc.sync.dma_start(out=st_f, in_=src_t)
                if after is not None:
                    add_dep_helper(ld.ins, after.ins, sync=False,
                                   reason="prologue piece ordering")
                st_b = stage.tile([P, rpp, D], BF16, tag=f"{name}sb", name=f"{name}sb", bufs=4 if name == "q" else 3)
                nc.gpsimd.tensor_copy(st_b, st_f)
                sd_t = sd[rows, :].rearrange("(p r) d -> p r d", r=rpp)
                st = nc.sync.dma_start(out=sd_t[:, :, 0:D], in_=st_b)
                tr = nc.sync.dma_start_transpose(out=dst[:, rows], in_=sd[rows, :])
                return st, tr

            def v_piece(row0, nrows, after=None):
                vch = slice(row0 // P, (row0 + nrows) // P)
                npc = nrows // P
                v_f = stage.tile([P, npc, D + 1], F32, tag="vf", name="vf")
                ld = nc.sync.dma_start(
                    out=v_f[:, 0:npc, :][:, :, 0:D], in_=v_tiled[:, vch, :]
                )
                if after is not None:
                    add_dep_helper(ld.ins, after.ins, sync=False,
                                   reason="prologue piece ordering")
                nc.vector.memset(v_f[:, 0:npc, D : D + 1], 1.0)
                nc.gpsimd.tensor_copy(v2[:, vch, :], v_f[:, 0:npc, :])

            def qk_piece_pe(name, src, dst, row0, nrows):
                # contiguous load: partition p <- rows [row0+p*rpp, ...).
                # The r-th slice [:, r, :] = rows {row0 + p*rpp + r} is one
                # PE-transpose unit whose output columns land strided (step
                # rpp) in dst.
                rpp = nrows // P
                src_t = src.ap()[row0 : row0 + nrows, :].rearrange(
                    "(p r) d -> p r d", r=rpp
                )
                st_f = stage.tile([P, rpp, D], F32, tag=f"{name}pf", name=f"{name}pf")
                ld = nc.sync.dma_start(out=st_f, in_=src_t)
                dst_v = dst[0:D, row0 : row0 + nrows].rearrange(
                    "d (p r) -> d r p", r=rpp
                )
                # quads of transposes share one PSUM slot; one copy per quad
                # (the copies are latency-dominated on DVE otherwise)
                for r0 in range(0, rpp, 4):
                    nq = min(4, rpp - r0)
                    tp = partp.tile([D, 4, P], F32, tag="part", name="tp")
                    for i in range(nq):
                        # fp32 PE transpose straight from the f32 staging;
                        # the PSUM->SBUF copy does the bf16 cast.
                        nc.tensor.transpose(tp[:, i, :], st_f[:, r0 + i, :], id_f32)
                    nc.vector.tensor_copy(
                        dst_v[:, r0 : r0 + nq, :], tp[:, 0:nq, :]
                    )
                return ld

            qk_piece_pe("q", q_dram, qt_all, *q_pe[0])
            qk_piece_pe("k", k_dram, kt_all, *k_pe[0])
            v_piece(*v_pieces[0])
            gate = None
            for pc in q_pe[1:]:
                gate = qk_piece_pe("q", q_dram, qt_all, *pc)
            for i, pc in enumerate(k_dma):
                _, ktr = qk_piece("k", k_dram, ksd, kt_all, *pc, after=gate)
                v_piece(*v_pieces[1 + i], after=gate)
                gate = ktr

            # ---------------- main loop: outer m-groups, inner n-blocks ---
            # (iters>1 repeats the whole main loop inside one NEFF for
            #  device-time measurement; results are idempotent)
            n_groups = len(gsizes)
            for _rep in range(iters):
              mc = 0
              for gi, g in enumerate(gsizes):
                  last_group = gi == n_groups - 1
                  for j in range(n_blocks):
                      ncol = slice(j * nb, (j + 1) * nb)
                      it = gi * n_blocks + j
                      if _rep == 0 and it in q_weave:
                          qk_piece_pe("q", q_dram, qt_all, *q_weave[it])
                      pool = sgpa if it % 2 == 0 else sgpb
                      s_g = pool.tile(
                          [P, g, nb], F32, tag=f"sg{it % 2}", name=f"sg{it % 2}"
                      )
                      for ci in range(g):
                          cc = mc + ci
                          nc.tensor.matmul(
                              s_g[:, ci, :],
                              kt_all[0:D, cc * P : (cc + 1) * P],
                              qt_all[0:D, ncol],
                              start=True,
                              stop=True,
                          )
                      p_g = pexp.tile([P, g, nb], BF16, tag="pg")
                      nc.scalar.activation(
                          out=p_g,
                          in_=s_g,
                          func=mybir.ActivationFunctionType.Exp,
                          scale=float(scale),
                      )
                      part = partp.tile([P, ntiles_blk, P], F32, tag="part")
                      for ci in range(g):
                          cc = mc + ci
                          for t in range(ntiles_blk):
                              nc.tensor.matmul(
                                  part[:, t, 0 : D + 1],
                                  p_g[:, ci, t * P : (t + 1) * P],
                                  v2[:, cc, :],
                                  start=(ci == 0 and t == 0),
                                  stop=(ci == g - 1 and t == ntiles_blk - 1),
                                  skip_group_check=True,
                              )
                      if gi == 0:
                          nc.vector.tensor_copy(oaccs[j], part[:, :, 0 : D + 1])
                      else:
                          nc.vector.tensor_add(
                              oaccs[j], oaccs[j], part[:, :, 0 : D + 1]
                          )

                      if last_group:
                          # epilogue for block j, streams with the inner loop
                          rinv = small.tile([P, ntiles_blk, 1], F32, tag="rinv")
                          nc.vector.reciprocal(rinv, oaccs[j][:, :, D : D + 1])
                          o_sb = outp.tile([P, ntiles_blk, D], F32, tag="osb")
                          rinv_b = bass.AP(
                              tensor=rinv.tensor,
                              offset=rinv.offset,
                              ap=[rinv.ap[0], rinv.ap[1], [0, D]],
                          )
                          nc.vector.tensor_tensor(
                              out=o_sb,
                              in0=oaccs[j][:, :, 0:D],
                              in1=rinv_b,
                              op=mybir.AluOpType.mult,
                          )
                          nc.sync.dma_start(
                              out=o_tiled[:, j * ntiles_blk : (j + 1) * ntiles_blk, :],
                              in_=o_sb,
                          )
                  mc += g

    nc.compile()
    return nc


_NC_CACHE = {}


def _get_nc(**kw):
    key = tuple(sorted(kw.items()))
    if key not in _NC_CACHE:
        _NC_CACHE[key] = build_nc(**kw)
    return _NC_CACHE[key]


def kernel(queries, keys, values, **run_kwargs):
    """Full-input entry point: [8, 4096, 64] fp32 each -> [8, 4096, 64] fp32."""
    nc = _get_nc()
    in_maps = [
        {
            "queries": np.ascontiguousarray(queries[b], dtype=np.float32),
            "keys": np.ascontiguousarray(keys[b], dtype=np.float32),
            "values": np.ascontiguousarray(values[b], dtype=np.float32),
        }
        for b in range(B)
    ]
    res = run_bass_kernel_spmd(nc, in_maps, core_ids=list(range(B)), **run_kwargs)
    out = np.stack([res.results[b]["out"] for b in range(B)]).astype(np.float32)
    if run_kwargs:
        kernel.last_results = res
    return out



# revision 6
# speedup vs baseline: 1.3813x; 1.3813x over previous
"""Trainium2 Bass kernel: batched dot-product attention.

Problem: B=8, N=M=4096, D=64, fp32.
  out[b] = softmax(Q[b] @ K[b].T / sqrt(D)) @ V[b]

Sharding: batch b -> core b (8 cores, no communication).

Per-core algorithm (flash attention, S^T layout). The 16.7M-exponential
softmax is the hard floor; it is split between two engines:
  - ACT (scalar) computes exp via the hardware spline:
      p = exp(scale*Y + bias), Y the biased score (below).
  - DVE (vector) computes the SAME values via a custom single-pass
    8-slice op ("EXP2_BITS_ANT") that emits the bf16 BIT PATTERN of
    2^y directly into an int16 view:
      out_bits = (Y + C2) + S*(Y - rne128(Y))^2
    where Y = 128*(y + 127) - 64 is produced by the score matmul itself:
    Q^T/K^T are pre-scaled by lambda = sqrt(16*log2(e)) and carry an
    extra contraction row (128.0 x 126.5) that adds the +16192 bias.
    rne128 comes from the fp32 magic-constant trick (+/-1.5*2^30); the
    quadratic term is the mantissa correction 2^f-1-f, max err ~0.3%,
    below bf16 quantization.  Constant factors common to both paths
    cancel in the softmax normalization.

Loop structure: keys split in two halves of 2048; per half, over the 8
query blocks (nb=512), over m-chunk groups (3,3,3,3,2,2); PV partials
accumulate in PSUM across the half's 16 chunks (V' carries a ones
column -> col 64 = softmax denominator), so the vector engine does no
per-group accumulation.  Half 0 copies partial->SBUF; half 1 adds,
reciprocals, scales and stores.

Prologue: early Q/K pieces are PE-transposed (scaled identity bakes in
lambda); the tails bounce through a bf16 DRAM staging [rows,128] (col 64
= bias row) and one hardware DMA-transpose each; V is strided-loaded
into [128, chunk, 65] with ones in column 64.
"""

import sys

import numpy as np

if "/opt/trn_rl_repo" not in sys.path:
    sys.path.insert(0, "/opt/trn_rl_repo")

import concourse.bass as bass
import concourse.tile as tile
from concourse import bacc, mybir
from concourse.tile import add_dep_helper
from concourse.bass_utils import run_bass_kernel_spmd
from concourse.masks import make_identity

B = 8
SEQ = 4096
D = 64
P = 128

F32 = mybir.dt.float32
BF16 = mybir.dt.bfloat16
I16 = mybir.dt.int16

# ---- exp2-bits constants -------------------------------------------------
LOG2E = float(np.log2(np.e))
LAMBDA = float(np.sqrt(16.0 * LOG2E))        # per-operand score scale
DQ = 128.0                                   # bias-row factors: DQ*DK = 16192
DK = 126.5                                   # = 127*128 - 64
K128 = float(1.5 * 2**30)                    # fp32 magic: ulp = 128
S_COEF = 0.00267                             # mantissa-correction quadratic
C2_COEF = 64.0 - 4096.0 * S_COEF + 0.554     # +64 unbias, residual centering
ACT_SCALE = float(np.log(2.0) / 128.0)
ACT_BIAS = float(-16192.0 * np.log(2.0) / 128.0)


def register_exp2_op():
    import concourse.dve_ops as dve_ops
    from concourse.dve_spec import Spec, Src0, C0, C1, C2

    for op in dve_ops.OPS:
        if op.name == "EXP2_BITS_ANT":
            return op

    _t = Src0 + C0
    _u = _t - C0
    _f = Src0 - _u
    body = (Src0 + C2) + (_f * _f) * C1

    def _ref(in0, in1, c0, c1, c2):
        y = np.asarray(in0, np.float32)
        t = (y + np.float32(c0)).astype(np.float32)
        u = (t - np.float32(c0)).astype(np.float32)
        f = (y - u).astype(np.float32)
        return (y + np.float32(c2)) + (f * f) * np.float32(c1)

    op = dve_ops.DveOp(
        "EXP2_BITS_ANT",
        Spec(body=body, reference=_ref),
        subdim=False,
        uops_sha={"v3": "ac9965176d749c87", "v4": "871a78a1589accca"},
    )
    dve_ops.OPS.append(op)
    dve_ops.CUSTOM_DVE_SPECS[op.name] = op.spec
    dve_ops._SUB_OPCODE_FOR_NAME[op.name] = (
        dve_ops._CUSTOM_DVE_ROW_BASE + len(dve_ops.OPS) - 1
    )
    return op


EXP2_OP = register_exp2_op()


def _groups(n_chunks, g):
    out, c = [], n_chunks
    while c > 0:
        out.append(min(g, c))
        c -= out[-1]
    return out


def build_nc(seq=SEQ, nb=512, iters=1, act_frac=0.56):
    n_mchunks = seq // P          # 32
    half_chunks = n_mchunks // 2  # 16
    n_blocks = seq // nb          # 8
    ntiles = nb // P              # 4

    nc = bacc.Bacc("TRN2", target_bir_lowering=False, debug=False)

    q_dram = nc.dram_tensor("queries", [seq, D], F32, kind="ExternalInput")
    k_dram = nc.dram_tensor("keys", [seq, D], F32, kind="ExternalInput")
    v_dram = nc.dram_tensor("values", [seq, D], F32, kind="ExternalInput")
    o_dram = nc.dram_tensor("out", [seq, D], F32, kind="ExternalOutput")

    v_tiled = v_dram.ap().rearrange("(t p) d -> p t d", p=P)
    o_tiled = o_dram.ap().rearrange("(t p) d -> p t d", p=P)

    with tile.TileContext(nc) as tc:
        with (
            tc.tile_pool(name="persist", bufs=1) as persist,
            tc.tile_pool(name="stage", bufs=2) as stage,
            tc.tile_pool(name="dstage", bufs=1, space="DRAM") as dstage,
            tc.tile_pool(name="pexp", bufs=3) as pexp,
            tc.tile_pool(name="outp", bufs=3) as outp,
            tc.tile_pool(name="small", bufs=4) as small,
            tc.tile_pool(name="sga", bufs=1, space=bass.MemorySpace.PSUM) as sgpa,
            tc.tile_pool(name="sgb", bufs=1, space=bass.MemorySpace.PSUM) as sgpb,
            tc.tile_pool(name="part", bufs=2, space=bass.MemorySpace.PSUM) as partp,
        ):
            qt_all = persist.tile([P, seq], BF16, tag="qt")
            kt_all = persist.tile([P, seq], BF16, tag="kt")
            v2 = persist.tile([P, n_mchunks, D + 1], BF16, tag="v2")
            oaccs = [
                persist.tile([P, ntiles, D + 1], F32, tag=f"oa{j}", name=f"oa{j}")
                for j in range(n_blocks)
            ]
            qsd = dstage.tile([seq, P], BF16, tag="qsd")
            ksd = dstage.tile([seq, P], BF16, tag="ksd")
            id_f32 = persist.tile([P, P], F32, tag="idf32")
            make_identity(nc, id_f32)
            bias_t = persist.tile([P, 1], F32, tag="biast")
            nc.gpsimd.memset(bias_t, ACT_BIAS)

            # ---------------- prologue (graduated pieces) ----------------
            def qk_piece_pe(name, src, dst, bias_val, row0, nrows):
                """Rows [row0, row0+nrows) -> dst[0:65, row0:...] via PE
                transpose with lambda-scaled identity; staging col 64 holds
                bias_val/LAMBDA so dst partition 64 = bias_val."""
                rpp = nrows // P
                src_t = src.ap()[row0 : row0 + nrows, :].rearrange(
                    "(p r) d -> p r d", r=rpp
                )
                st_f = stage.tile([P, rpp, D + 1], F32, tag=f"{name}pf", name=f"{name}pf")
                ld = nc.sync.dma_start(out=st_f[:, :, 0:D], in_=src_t)
                nc.gpsimd.memset(st_f[:, :, D : D + 1], bias_val / LAMBDA)
                dst_v = dst[0 : D + 1, row0 : row0 + nrows].rearrange(
                    "d (p r) -> d r p", r=rpp
                )
                for r0 in range(0, rpp, 4):
                    nq = min(4, rpp - r0)
                    tp = partp.tile([D + 1, 4, P], F32, tag="part", name="tp")
                    for i in range(nq):
                        nc.tensor.transpose(
                            tp[:, i, :], st_f[:, r0 + i, :], id_f32
                        )
                    nc.vector.tensor_scalar_mul(
                        dst_v[:, r0 : r0 + nq, :], tp[:, 0:nq, :], LAMBDA
                    )
                return ld

            def qk_piece_dma(name, src, sd, dst, bias_val, row0, nrows, after=None):
                """Staged bf16 DMA-transpose path: load f32, Pool-cast with
                lambda scale + bias col, store staging, hw transpose."""
                rows = slice(row0, row0 + nrows)
                rpp = nrows // P
                src_t = src.ap()[rows, :].rearrange("(p r) d -> p r d", r=rpp)
                st_f = stage.tile([P, rpp, D], F32, tag=f"{name}sf", name=f"{name}sf", bufs=3)
                ld = nc.sync.dma_start(out=st_f, in_=src_t)
                if after is not None:
                    add_dep_helper(ld.ins, after.ins, sync=False,
                                   reason="prologue piece ordering")
                st_b = stage.tile([P, rpp, D + 1], BF16, tag=f"{name}sb", name=f"{name}sb", bufs=3)
                nc.gpsimd.tensor_scalar_mul(st_b[:, :, 0:D], st_f, LAMBDA)
                nc.gpsimd.memset(st_b[:, :, D : D + 1], bias_val)
                sd_t = sd[rows, :].rearrange("(p r) d -> p r d", r=rpp)
                st = nc.sync.dma_start(out=sd_t[:, :, 0 : D + 1], in_=st_b)
                tr = nc.sync.dma_start_transpose(out=dst[:, rows], in_=sd[rows, :])
                return st, tr

            def v_piece(row0, nrows, after=None):
                vch = slice(row0 // P, (row0 + nrows) // P)
                npc = nrows // P
                v_f = stage.tile([P, npc, D + 1], F32, tag="vf", name="vf", bufs=3)
                ld = nc.sync.dma_start(
                    out=v_f[:, 0:npc, :][:, :, 0:D], in_=v_tiled[:, vch, :]
                )
                if after is not None:
                    add_dep_helper(ld.ins, after.ins, sync=False,
                                   reason="prologue piece ordering")
                nc.gpsimd.memset(v_f[:, 0:npc, D : D + 1], 1.0)
                nc.gpsimd.tensor_copy(v2[:, vch, :], v_f[:, 0:npc, :])

            # PE path: K chunks 0-11 (rows 0-1535), Q block 0 (rows 0-511)
            qk_piece_pe("k", k_dram, kt_all, DK, 0, 512)
            v_piece(0, 1024)
            qk_piece_pe("q", q_dram, qt_all, DQ, 0, 512)
            qk_piece_pe("k", k_dram, kt_all, DK, 512, 512)
            qk_piece_pe("k", k_dram, kt_all, DK, 1024, 512)
            # DMA path, ordered by need time
            _, g1 = qk_piece_dma("k", k_dram, ksd, kt_all, DK, 1536, 512)
            v_piece(1024, 1024, after=g1)
            _, g2 = qk_piece_dma("q", q_dram, qsd, qt_all, DQ, 512, 1536, after=g1)
            _, g3 = qk_piece_dma("k", k_dram, ksd, kt_all, DK, 2048, 2048, after=g2)
            v_piece(2048, 2048, after=g2)
            _, g4 = qk_piece_dma("q", q_dram, qsd, qt_all, DQ, 2048, 2048, after=g3)

            # ---------------- main loop ----------------------------------
            gsizes = _groups(half_chunks, 3)   # (3,3,3,3,2,2)
            it = 0
            for _rep in range(iters):
              for h in range(2):
                for j in range(n_blocks):
                    ncol = slice(j * nb, (j + 1) * nb)
                    partial = partp.tile([P, ntiles, P], F32, tag="part", name="pt")
                    mc = h * half_chunks
                    n_g = len(gsizes)
                    for gi, g in enumerate(gsizes):
                        pool = sgpa if it % 2 == 0 else sgpb
                        s_g = pool.tile(
                            [P, g, nb], F32, tag=f"sg{it % 2}", name=f"sg{it % 2}"
                        )
                        it += 1
                        for ci in range(g):
                            cc = mc + ci
                            nc.tensor.matmul(
                                s_g[:, ci, :],
                                kt_all[0 : D + 1, cc * P : (cc + 1) * P],
                                qt_all[0 : D + 1, ncol],
                                start=True,
                                stop=True,
                            )
                        E = g * nb
                        a = int(E * act_frac) // 32 * 32
                        s_flat = s_g.rearrange("p g n -> p (g n)")
                        p_g = pexp.tile([P, g, nb], BF16, tag="pg", name="pg")
                        p_flat = p_g.rearrange("p g n -> p (g n)")
                        nc.scalar.activation(
                            out=p_flat[:, 0:a],
                            in_=s_flat[:, 0:a],
                            func=mybir.ActivationFunctionType.Exp,
                            scale=ACT_SCALE,
                            bias=bias_t,
                        )
                        nc.vector._custom_dve(
                            EXP2_OP,
                            out=p_flat[:, a:E].bitcast(I16),
                            in0=s_flat[:, a:E],
                            s0=K128,
                            s1=S_COEF,
                            imm2=C2_COEF,
                        )
                        for ci in range(g):
                            cc = mc + ci
                            for t in range(ntiles):
                                nc.tensor.matmul(
                                    partial[:, t, 0 : D + 1],
                                    p_g[:, ci, t * P : (t + 1) * P],
                                    v2[:, cc, :],
                                    start=(gi == 0 and ci == 0 and t == 0),
                                    stop=(gi == n_g - 1 and ci == g - 1 and t == ntiles - 1),
                                    skip_group_check=True,
                                )
                        mc += g

                    if h == 0:
                        nc.scalar.activation(
                            out=oaccs[j],
                            in_=partial[:, :, 0 : D + 1],
                            func=mybir.ActivationFunctionType.Copy,
                            scale=1.0,
                        )
                    else:
                        osum = outp.tile([P, ntiles, D + 1], F32, tag="osum", name="osum")
                        nc.vector.tensor_add(osum, oaccs[j], partial[:, :, 0 : D + 1])
                        rinv = small.tile([P, ntiles, 1], F32, tag="rinv", name="rinv")
                        nc.vector.reciprocal(rinv, osum[:, :, D : D + 1])
                        o_sb = outp.tile([P, ntiles, D], F32, tag="osb", name="osb")
                        rinv_b = bass.AP(
                            tensor=rinv.tensor,
                            offset=rinv.offset,
                            ap=[rinv.ap[0], rinv.ap[1], [0, D]],
                        )
                        nc.vector.tensor_tensor(
                            out=o_sb,
                            in0=osum[:, :, 0:D],
                            in1=rinv_b,
                            op=mybir.AluOpType.mult,
                        )
                        nc.sync.dma_start(
                            out=o_tiled[:, j * ntiles : (j + 1) * ntiles, :],
                            in_=o_sb,
                        )

    nc.compile()
    return nc


_NC_CACHE = {}


def _get_nc(**kw):
    key = tuple(sorted(kw.items()))
    if key not in _NC_CACHE:
        _NC_CACHE[key] = build_nc(**kw)
    return _NC_CACHE[key]


def kernel(queries, keys, values, **run_kwargs):
    """Full-input entry point: [8, 4096, 64] fp32 each -> [8, 4096, 64] fp32."""
    nc = _get_nc()
    in_maps = [
        {
            "queries": np.ascontiguousarray(queries[b], dtype=np.float32),
            "keys": np.ascontiguousarray(keys[b], dtype=np.float32),
            "values": np.ascontiguousarray(values[b], dtype=np.float32),
        }
        for b in range(B)
    ]
    res = run_bass_kernel_spmd(nc, in_maps, core_ids=list(range(B)), **run_kwargs)
    out = np.stack([res.results[b]["out"] for b in range(B)]).astype(np.float32)
    if run_kwargs:
        kernel.last_results = res
    return out


# revision 9
# speedup vs baseline: 4.4004x; 3.1856x over previous
"""Trainium2 Bass kernel: batched dot-product attention.

Problem: B=8, N=M=4096, D=64, fp32.
  out[b] = softmax(Q[b] @ K[b].T / sqrt(D)) @ V[b]

Sharding: batch b -> core b (8 cores, no communication).

Per-core algorithm (flash attention, S^T layout). The 16.7M-exponential
softmax is the hard floor; it is split between two engines:
  - ACT (scalar) computes exp via the hardware spline:
      p = exp(scale*Y + bias), Y the biased score (below).
  - DVE (vector) computes the SAME values via a custom single-pass
    8-slice op ("EXP2_BITS_ANT") that emits the bf16 BIT PATTERN of
    2^y directly into an int16 view:
      out_bits = (Y + C2) + S*(Y - rne128(Y))^2
    where Y = 128*(y + 127) - 64 is produced by the score matmul itself:
    Q^T/K^T are pre-scaled by lambda = sqrt(16*log2(e)) and carry an
    extra contraction row (128.0 x 126.5) that adds the +16192 bias.
    rne128 comes from the fp32 magic-constant trick (+/-1.5*2^30); the
    quadratic term is the mantissa correction 2^f-1-f, max err ~0.3%,
    below bf16 quantization.  Constant factors common to both paths
    cancel in the softmax normalization.

Loop structure: keys split in two halves of 2048; per half, over the 8
query blocks (nb=512), over m-chunk groups (3,3,3,3,2,2); PV partials
accumulate in PSUM across the half's 16 chunks (V' carries a ones
column -> col 64 = softmax denominator), so the vector engine does no
per-group accumulation.  Half 0 copies partial->SBUF; half 1 adds,
reciprocals, scales and stores.

Prologue: early Q/K pieces are PE-transposed (scaled identity bakes in
lambda); the tails bounce through a bf16 DRAM staging [rows,128] (col 64
= bias row) and one hardware DMA-transpose each; V is strided-loaded
into [128, chunk, 65] with ones in column 64.
"""

import sys

import numpy as np

if "/opt/trn_rl_repo" not in sys.path:
    sys.path.insert(0, "/opt/trn_rl_repo")

import concourse.bass as bass
import concourse.tile as tile
from concourse import bacc, mybir
from concourse.tile import add_dep_helper
from concourse.bass_utils import run_bass_kernel_spmd
from concourse.masks import make_identity

B = 8
SEQ = 4096
D = 64
P = 128

F32 = mybir.dt.float32
BF16 = mybir.dt.bfloat16
I16 = mybir.dt.int16

# ---- exp2-bits constants -------------------------------------------------
LOG2E = float(np.log2(np.e))
LAMBDA = float(np.sqrt(16.0 * LOG2E))        # per-operand score scale
DQ = 128.0                                   # bias-row factors: DQ*DK = 16192
DK = 126.5                                   # = 127*128 - 64
K128 = float(1.5 * 2**30)                    # fp32 magic: ulp = 128
S_COEF = 0.00267                             # mantissa-correction quadratic
C2_COEF = 64.0 - 4096.0 * S_COEF + 0.554     # +64 unbias, residual centering
ACT_SCALE = float(np.log(2.0) / 128.0)
ACT_BIAS = float(-16192.0 * np.log(2.0) / 128.0)


def register_exp2_op():
    import concourse.dve_ops as dve_ops
    from concourse.dve_spec import Spec, Src0, C0, C1, C2

    for op in dve_ops.OPS:
        if op.name == "EXP2_BITS_ANT":
            return op

    _t = Src0 + C0
    _u = _t - C0
    _f = Src0 - _u
    body = (Src0 + C2) + (_f * _f) * C1

    def _ref(in0, in1, c0, c1, c2):
        y = np.asarray(in0, np.float32)
        t = (y + np.float32(c0)).astype(np.float32)
        u = (t - np.float32(c0)).astype(np.float32)
        f = (y - u).astype(np.float32)
        return (y + np.float32(c2)) + (f * f) * np.float32(c1)

    op = dve_ops.DveOp(
        "EXP2_BITS_ANT",
        Spec(body=body, reference=_ref),
        subdim=False,
        uops_sha={"v3": "ac9965176d749c87", "v4": "871a78a1589accca"},
    )
    dve_ops.OPS.append(op)
    dve_ops.CUSTOM_DVE_SPECS[op.name] = op.spec
    dve_ops._SUB_OPCODE_FOR_NAME[op.name] = (
        dve_ops._CUSTOM_DVE_ROW_BASE + len(dve_ops.OPS) - 1
    )
    return op


EXP2_OP = register_exp2_op()


def _groups(n_chunks, g):
    out, c = [], n_chunks
    while c > 0:
        out.append(min(g, c))
        c -= out[-1]
    return out


def build_nc(seq=SEQ, nb=512, iters=1, act_frac=0.56, pe_rows_k=1536):
    n_mchunks = seq // P          # 32
    half_chunks = n_mchunks // 2  # 16
    n_blocks = seq // nb          # 8
    ntiles = nb // P              # 4

    nc = bacc.Bacc("TRN2", target_bir_lowering=False, debug=False)

    q_dram = nc.dram_tensor("queries", [seq, D], F32, kind="ExternalInput")
    k_dram = nc.dram_tensor("keys", [seq, D], F32, kind="ExternalInput")
    v_dram = nc.dram_tensor("values", [seq, D], F32, kind="ExternalInput")
    o_dram = nc.dram_tensor("out", [seq, D], F32, kind="ExternalOutput")

    v_tiled = v_dram.ap().rearrange("(t p) d -> p t d", p=P)
    o_tiled = o_dram.ap().rearrange("(t p) d -> p t d", p=P)

    with tile.TileContext(nc) as tc:
        with (
            tc.tile_pool(name="persist", bufs=1) as persist,
            tc.tile_pool(name="stage", bufs=2) as stage,
            tc.tile_pool(name="dstage", bufs=1, space="DRAM") as dstage,
            tc.tile_pool(name="pexp", bufs=3) as pexp,
            tc.tile_pool(name="outp", bufs=3) as outp,
            tc.tile_pool(name="small", bufs=4) as small,
            tc.tile_pool(name="sga", bufs=1, space=bass.MemorySpace.PSUM) as sgpa,
            tc.tile_pool(name="sgb", bufs=1, space=bass.MemorySpace.PSUM) as sgpb,
            tc.tile_pool(name="part", bufs=2, space=bass.MemorySpace.PSUM) as partp,
        ):
            qt_all = persist.tile([P, seq], BF16, tag="qt")
            kt_all = persist.tile([P, seq], BF16, tag="kt")
            v2 = persist.tile([P, n_mchunks, D + 1], BF16, tag="v2")
            oaccs = [
                persist.tile([P, ntiles, D + 1], F32, tag=f"oa{j}", name=f"oa{j}")
                for j in range(n_blocks)
            ]
            qsd = dstage.tile([seq, P], BF16, tag="qsd")
            ksd = dstage.tile([seq, P], BF16, tag="ksd")
            id_f32 = persist.tile([P, P], F32, tag="idf32")
            make_identity(nc, id_f32)
            bias_t = persist.tile([P, 1], F32, tag="biast")
            nc.gpsimd.memset(bias_t, ACT_BIAS)

            # ---------------- prologue (graduated pieces) ----------------
            def qk_piece_pe(name, src, dst, bias_val, row0, nrows):
                """Rows [row0, row0+nrows) -> dst[0:65, row0:...] via PE
                transpose with lambda-scaled identity; staging col 64 holds
                bias_val/LAMBDA so dst partition 64 = bias_val."""
                rpp = nrows // P
                src_t = src.ap()[row0 : row0 + nrows, :].rearrange(
                    "(p r) d -> p r d", r=rpp
                )
                st_f = stage.tile([P, rpp, D + 1], F32, tag=f"{name}pf", name=f"{name}pf")
                ld = nc.sync.dma_start(out=st_f[:, :, 0:D], in_=src_t)
                nc.gpsimd.memset(st_f[:, :, D : D + 1], bias_val / LAMBDA)
                dst_v = dst[0 : D + 1, row0 : row0 + nrows].rearrange(
                    "d (p r) -> d r p", r=rpp
                )
                for r0 in range(0, rpp, 4):
                    nq = min(4, rpp - r0)
                    tp = partp.tile([D + 1, 4, P], F32, tag="part", name="tp")
                    for i in range(nq):
                        nc.tensor.transpose(
                            tp[:, i, :], st_f[:, r0 + i, :], id_f32
                        )
                    nc.vector.tensor_scalar_mul(
                        dst_v[:, r0 : r0 + nq, :], tp[:, 0:nq, :], LAMBDA
                    )
                return ld

            def qk_piece_dma(name, src, sd, dst, bias_val, row0, nrows, after=None):
                """Staged bf16 DMA-transpose path: load f32, Pool-cast with
                lambda scale + bias col, store staging, hw transpose."""
                rows = slice(row0, row0 + nrows)
                rpp = nrows // P
                src_t = src.ap()[rows, :].rearrange("(p r) d -> p r d", r=rpp)
                st_f = stage.tile([P, rpp, D], F32, tag=f"{name}sf", name=f"{name}sf", bufs=3)
                ld = nc.sync.dma_start(out=st_f, in_=src_t)
                if after is not None:
                    add_dep_helper(ld.ins, after.ins, sync=False,
                                   reason="prologue piece ordering")
                st_b = stage.tile([P, rpp, D + 1], BF16, tag=f"{name}sb", name=f"{name}sb", bufs=3)
                nc.gpsimd.tensor_scalar_mul(st_b[:, :, 0:D], st_f, LAMBDA)
                nc.gpsimd.memset(st_b[:, :, D : D + 1], bias_val)
                sd_t = sd[rows, :].rearrange("(p r) d -> p r d", r=rpp)
                st = nc.sync.dma_start(out=sd_t[:, :, 0 : D + 1], in_=st_b)
                tr = nc.sync.dma_start_transpose(out=dst[:, rows], in_=sd[rows, :])
                return st, tr

            def v_piece(row0, nrows, after=None):
                vch = slice(row0 // P, (row0 + nrows) // P)
                npc = nrows // P
                v_f = stage.tile([P, npc, D + 1], F32, tag="vf", name="vf", bufs=3)
                ld = nc.sync.dma_start(
                    out=v_f[:, 0:npc, :][:, :, 0:D], in_=v_tiled[:, vch, :]
                )
                if after is not None:
                    add_dep_helper(ld.ins, after.ins, sync=False,
                                   reason="prologue piece ordering")
                nc.gpsimd.memset(v_f[:, 0:npc, D : D + 1], 1.0)
                nc.gpsimd.tensor_copy(v2[:, vch, :], v_f[:, 0:npc, :])

            # PE path: K rows [0, pe_rows_k), Q block 0 (rows 0-511)
            qk_piece_pe("k", k_dram, kt_all, DK, 0, 512)
            v_piece(0, 1024)
            qk_piece_pe("q", q_dram, qt_all, DQ, 0, 512)
            for r0 in range(512, pe_rows_k, 512):
                qk_piece_pe("k", k_dram, kt_all, DK, r0, 512)
            # DMA path, ordered by need time
            _, g1 = qk_piece_dma("k", k_dram, ksd, kt_all, DK,
                                 pe_rows_k, 2048 - pe_rows_k)
            v_piece(1024, 1024, after=g1)
            _, g2 = qk_piece_dma("q", q_dram, qsd, qt_all, DQ, 512, 1536, after=g1)
            _, g3 = qk_piece_dma("k", k_dram, ksd, kt_all, DK, 2048, 2048, after=g2)
            v_piece(2048, 2048, after=g2)
            _, g4 = qk_piece_dma("q", q_dram, qsd, qt_all, DQ, 2048, 2048, after=g3)

            # ---------------- main loop ----------------------------------
            gsizes = _groups(half_chunks, 3)   # (3,3,3,3,2,2)
            it = 0
            for _rep in range(iters):
              for h in range(2):
                for j in range(n_blocks):
                    ncol = slice(j * nb, (j + 1) * nb)
                    partial = partp.tile([P, ntiles, P], F32, tag="part", name="pt")
                    mc = h * half_chunks
                    n_g = len(gsizes)
                    for gi, g in enumerate(gsizes):
                        pool = sgpa if it % 2 == 0 else sgpb
                        s_g = pool.tile(
                            [P, g, nb], F32, tag=f"sg{it % 2}", name=f"sg{it % 2}"
                        )
                        it += 1
                        for ci in range(g):
                            cc = mc + ci
                            nc.tensor.matmul(
                                s_g[:, ci, :],
                                kt_all[0 : D + 1, cc * P : (cc + 1) * P],
                                qt_all[0 : D + 1, ncol],
                                start=True,
                                stop=True,
                            )
                        E = g * nb
                        a = int(E * act_frac) // 32 * 32
                        s_flat = s_g.rearrange("p g n -> p (g n)")
                        p_g = pexp.tile([P, g, nb], BF16, tag="pg", name="pg")
                        p_flat = p_g.rearrange("p g n -> p (g n)")
                        nc.scalar.activation(
                            out=p_flat[:, 0:a],
                            in_=s_flat[:, 0:a],
                            func=mybir.ActivationFunctionType.Exp,
                            scale=ACT_SCALE,
                            bias=bias_t,
                        )
                        nc.vector._custom_dve(
                            EXP2_OP,
                            out=p_flat[:, a:E].bitcast(I16),
                            in0=s_flat[:, a:E],
                            s0=K128,
                            s1=S_COEF,
                            imm2=C2_COEF,
                        )
                        for ci in range(g):
                            cc = mc + ci
                            for t in range(ntiles):
                                nc.tensor.matmul(
                                    partial[:, t, 0 : D + 1],
                                    p_g[:, ci, t * P : (t + 1) * P],
                                    v2[:, cc, :],
                                    start=(gi == 0 and ci == 0 and t == 0),
                                    stop=(gi == n_g - 1 and ci == g - 1 and t == ntiles - 1),
                                    skip_group_check=True,
                                )
                        mc += g

                    if h == 0:
                        nc.scalar.activation(
                            out=oaccs[j],
                            in_=partial[:, :, 0 : D + 1],
                            func=mybir.ActivationFunctionType.Copy,
                            scale=1.0,
                        )
                    else:
                        # partial -> SBUF on ACT; add + broadcast-mult on the
                        # (otherwise idle) Pool engine; DVE only does the
                        # small reciprocal.
                        ptmp = outp.tile([P, ntiles, D + 1], F32, tag="ptmp", name="ptmp")
                        nc.scalar.activation(
                            out=ptmp,
                            in_=partial[:, :, 0 : D + 1],
                            func=mybir.ActivationFunctionType.Copy,
                            scale=1.0,
                        )
                        osum = outp.tile([P, ntiles, D + 1], F32, tag="osum", name="osum")
                        nc.gpsimd.tensor_add(osum, oaccs[j], ptmp)
                        rinv = small.tile([P, ntiles, 1], F32, tag="rinv", name="rinv")
                        nc.vector.reciprocal(rinv, osum[:, :, D : D + 1])
                        o_sb = outp.tile([P, ntiles, D], F32, tag="osb", name="osb")
                        rinv_b = bass.AP(
                            tensor=rinv.tensor,
                            offset=rinv.offset,
                            ap=[rinv.ap[0], rinv.ap[1], [0, D]],
                        )
                        nc.gpsimd.tensor_tensor(
                            out=o_sb,
                            in0=osum[:, :, 0:D],
                            in1=rinv_b,
                            op=mybir.AluOpType.mult,
                        )
                        nc.sync.dma_start(
                            out=o_tiled[:, j * ntiles : (j + 1) * ntiles, :],
                            in_=o_sb,
                        )

    nc.compile()
    return nc


_NC_CACHE = {}


def _get_nc(**kw):
    key = tuple(sorted(kw.items()))
    if key not in _NC_CACHE:
        _NC_CACHE[key] = build_nc(**kw)
    return _NC_CACHE[key]


def kernel(queries, keys, values, **run_kwargs):
    """Full-input entry point: [8, 4096, 64] fp32 each -> [8, 4096, 64] fp32."""
    nc = _get_nc()
    in_maps = [
        {
            "queries": np.ascontiguousarray(queries[b], dtype=np.float32),
            "keys": np.ascontiguousarray(keys[b], dtype=np.float32),
            "values": np.ascontiguousarray(values[b], dtype=np.float32),
        }
        for b in range(B)
    ]
    res = run_bass_kernel_spmd(nc, in_maps, core_ids=list(range(B)), **run_kwargs)
    out = np.stack([res.results[b]["out"] for b in range(B)]).astype(np.float32)
    if run_kwargs:
        kernel.last_results = res
    return out


# revision 11
# speedup vs baseline: 7.5150x; 1.7078x over previous
"""Trainium2 Bass kernel: batched dot-product attention.

Problem: B=8, N=M=4096, D=64, fp32.
  out[b] = softmax(Q[b] @ K[b].T / sqrt(D)) @ V[b]

Sharding: batch b -> core b (8 cores, no communication).

Per-core algorithm (flash attention, S^T layout). The 16.7M-exponential
softmax is the hard floor; it is split between two engines:
  - ACT (scalar) computes exp via the hardware spline:
      p = exp(scale*Y + bias), Y the biased score (below).
  - DVE (vector) computes the SAME values via a custom single-pass
    8-slice op ("EXP2_BITS_ANT") that emits the bf16 BIT PATTERN of
    2^y directly into an int16 view:
      out_bits = (Y + C2) + S*(Y - rne128(Y))^2
    where Y = 128*(y + 127) - 64 is produced by the score matmul itself:
    Q^T/K^T are pre-scaled by lambda = sqrt(16*log2(e)) and carry an
    extra contraction row (128.0 x 126.5) that adds the +16192 bias.
    rne128 comes from the fp32 magic-constant trick (+/-1.5*2^30); the
    quadratic term is the mantissa correction 2^f-1-f, max err ~0.3%,
    below bf16 quantization.  Constant factors common to both paths
    cancel in the softmax normalization.

Loop structure: keys split in two halves of 2048; per half, over the 8
query blocks (nb=512), over m-chunk groups (3,3,3,3,2,2); PV partials
accumulate in PSUM across the half's 16 chunks (V' carries a ones
column -> col 64 = softmax denominator), so the vector engine does no
per-group accumulation.  Half 0 copies partial->SBUF (ACT); half 1:
ACT copies the second partial out of PSUM, the otherwise-idle Pool
engine does the add and the broadcast multiply by 1/denominator, DVE
only the small reciprocal.  Engine budget per core (balanced):
PE ~88-92us (S matmuls stream-bound, PV matmuls dispatch-bound),
ACT ~78us, DVE ~78us; measured 94.6us/core steady state vs 130.6us
for the all-ACT baseline.

Prologue: early Q/K pieces are PE-transposed (scaled identity bakes in
lambda); the tails bounce through a bf16 DRAM staging [rows,128] (col 64
= bias row) and one hardware DMA-transpose each; V is strided-loaded
into [128, chunk, 65] with ones in column 64.
"""

import sys

import numpy as np

if "/opt/trn_rl_repo" not in sys.path:
    sys.path.insert(0, "/opt/trn_rl_repo")

import concourse.bass as bass
import concourse.tile as tile
from concourse import bacc, mybir
from concourse.tile import add_dep_helper
from concourse.bass_utils import run_bass_kernel_spmd
from concourse.masks import make_identity

B = 8
SEQ = 4096
D = 64
P = 128

F32 = mybir.dt.float32
BF16 = mybir.dt.bfloat16
I16 = mybir.dt.int16

# ---- exp2-bits constants -------------------------------------------------
LOG2E = float(np.log2(np.e))
LAMBDA = float(np.sqrt(16.0 * LOG2E))        # per-operand score scale
DQ = 128.0                                   # bias-row factors: DQ*DK = 16192
DK = 126.5                                   # = 127*128 - 64
K128 = float(1.5 * 2**30)                    # fp32 magic: ulp = 128
S_COEF = 0.00267                             # mantissa-correction quadratic
C2_COEF = 64.0 - 4096.0 * S_COEF + 0.554     # +64 unbias, residual centering
ACT_SCALE = float(np.log(2.0) / 128.0)
ACT_BIAS = float(-16192.0 * np.log(2.0) / 128.0)


def register_exp2_op():
    import concourse.dve_ops as dve_ops
    from concourse.dve_spec import Spec, Src0, C0, C1, C2

    for op in dve_ops.OPS:
        if op.name == "EXP2_BITS_ANT":
            return op

    _t = Src0 + C0
    _u = _t - C0
    _f = Src0 - _u
    body = (Src0 + C2) + (_f * _f) * C1

    def _ref(in0, in1, c0, c1, c2):
        y = np.asarray(in0, np.float32)
        t = (y + np.float32(c0)).astype(np.float32)
        u = (t - np.float32(c0)).astype(np.float32)
        f = (y - u).astype(np.float32)
        return (y + np.float32(c2)) + (f * f) * np.float32(c1)

    op = dve_ops.DveOp(
        "EXP2_BITS_ANT",
        Spec(body=body, reference=_ref),
        subdim=False,
        uops_sha={"v3": "ac9965176d749c87", "v4": "871a78a1589accca"},
    )
    dve_ops.OPS.append(op)
    dve_ops.CUSTOM_DVE_SPECS[op.name] = op.spec
    dve_ops._SUB_OPCODE_FOR_NAME[op.name] = (
        dve_ops._CUSTOM_DVE_ROW_BASE + len(dve_ops.OPS) - 1
    )
    return op


EXP2_OP = register_exp2_op()


def _groups(n_chunks, g):
    out, c = [], n_chunks
    while c > 0:
        out.append(min(g, c))
        c -= out[-1]
    return out


def build_nc(seq=SEQ, nb=512, iters=1, act_frac=0.54, pe_rows_k=1536):
    n_mchunks = seq // P          # 32
    half_chunks = n_mchunks // 2  # 16
    n_blocks = seq // nb          # 8
    ntiles = nb // P              # 4

    nc = bacc.Bacc("TRN2", target_bir_lowering=False, debug=False)

    q_dram = nc.dram_tensor("queries", [seq, D], F32, kind="ExternalInput")
    k_dram = nc.dram_tensor("keys", [seq, D], F32, kind="ExternalInput")
    v_dram = nc.dram_tensor("values", [seq, D], F32, kind="ExternalInput")
    o_dram = nc.dram_tensor("out", [seq, D], F32, kind="ExternalOutput")

    v_tiled = v_dram.ap().rearrange("(t p) d -> p t d", p=P)
    o_tiled = o_dram.ap().rearrange("(t p) d -> p t d", p=P)

    with tile.TileContext(nc) as tc:
        with (
            tc.tile_pool(name="persist", bufs=1) as persist,
            tc.tile_pool(name="stage", bufs=2) as stage,
            tc.tile_pool(name="dstage", bufs=1, space="DRAM") as dstage,
            tc.tile_pool(name="pexp", bufs=3) as pexp,
            tc.tile_pool(name="outp", bufs=3) as outp,
            tc.tile_pool(name="small", bufs=4) as small,
            tc.tile_pool(name="sga", bufs=1, space=bass.MemorySpace.PSUM) as sgpa,
            tc.tile_pool(name="sgb", bufs=1, space=bass.MemorySpace.PSUM) as sgpb,
            tc.tile_pool(name="part", bufs=2, space=bass.MemorySpace.PSUM) as partp,
        ):
            qt_all = persist.tile([P, seq], BF16, tag="qt")
            kt_all = persist.tile([P, seq], BF16, tag="kt")
            v2 = persist.tile([P, n_mchunks, D + 1], BF16, tag="v2")
            oaccs = [
                persist.tile([P, ntiles, D + 1], F32, tag=f"oa{j}", name=f"oa{j}")
                for j in range(n_blocks)
            ]
            qsd = dstage.tile([seq, P], BF16, tag="qsd")
            ksd = dstage.tile([seq, P], BF16, tag="ksd")
            id_f32 = persist.tile([P, P], F32, tag="idf32")
            make_identity(nc, id_f32)
            bias_t = persist.tile([P, 1], F32, tag="biast")
            nc.gpsimd.memset(bias_t, ACT_BIAS)

            # ---------------- prologue (graduated pieces) ----------------
            def qk_piece_pe(name, src, dst, bias_val, row0, nrows):
                """Rows [row0, row0+nrows) -> dst[0:65, row0:...] via PE
                transpose with lambda-scaled identity; staging col 64 holds
                bias_val/LAMBDA so dst partition 64 = bias_val."""
                rpp = nrows // P
                src_t = src.ap()[row0 : row0 + nrows, :].rearrange(
                    "(p r) d -> p r d", r=rpp
                )
                st_f = stage.tile([P, rpp, D + 1], F32, tag=f"{name}pf", name=f"{name}pf")
                ld = nc.sync.dma_start(out=st_f[:, :, 0:D], in_=src_t)
                nc.gpsimd.memset(st_f[:, :, D : D + 1], bias_val / LAMBDA)
                dst_v = dst[0 : D + 1, row0 : row0 + nrows].rearrange(
                    "d (p r) -> d r p", r=rpp
                )
                for r0 in range(0, rpp, 4):
                    nq = min(4, rpp - r0)
                    tp = partp.tile([D + 1, 4, P], F32, tag="part", name="tp")
                    for i in range(nq):
                        nc.tensor.transpose(
                            tp[:, i, :], st_f[:, r0 + i, :], id_f32
                        )
                    nc.vector.tensor_scalar_mul(
                        dst_v[:, r0 : r0 + nq, :], tp[:, 0:nq, :], LAMBDA
                    )
                return ld

            def qk_piece_dma(name, src, sd, dst, bias_val, row0, nrows, after=None):
                """Staged bf16 DMA-transpose path: load f32, Pool-cast with
                lambda scale + bias col, store staging, hw transpose."""
                rows = slice(row0, row0 + nrows)
                rpp = nrows // P
                src_t = src.ap()[rows, :].rearrange("(p r) d -> p r d", r=rpp)
                st_f = stage.tile([P, rpp, D], F32, tag=f"{name}sf", name=f"{name}sf", bufs=3)
                ld = nc.sync.dma_start(out=st_f, in_=src_t)
                if after is not None:
                    add_dep_helper(ld.ins, after.ins, sync=False,
                                   reason="prologue piece ordering")
                st_b = stage.tile([P, rpp, D + 1], BF16, tag=f"{name}sb", name=f"{name}sb", bufs=3)
                nc.gpsimd.tensor_scalar_mul(st_b[:, :, 0:D], st_f, LAMBDA)
                nc.gpsimd.memset(st_b[:, :, D : D + 1], bias_val)
                sd_t = sd[rows, :].rearrange("(p r) d -> p r d", r=rpp)
                st = nc.sync.dma_start(out=sd_t[:, :, 0 : D + 1], in_=st_b)
                tr = nc.sync.dma_start_transpose(out=dst[:, rows], in_=sd[rows, :])
                return st, tr

            def v_piece(row0, nrows, after=None):
                vch = slice(row0 // P, (row0 + nrows) // P)
                npc = nrows // P
                v_f = stage.tile([P, npc, D + 1], F32, tag="vf", name="vf", bufs=3)
                ld = nc.sync.dma_start(
                    out=v_f[:, 0:npc, :][:, :, 0:D], in_=v_tiled[:, vch, :]
                )
                if after is not None:
                    add_dep_helper(ld.ins, after.ins, sync=False,
                                   reason="prologue piece ordering")
                nc.gpsimd.memset(v_f[:, 0:npc, D : D + 1], 1.0)
                nc.gpsimd.tensor_copy(v2[:, vch, :], v_f[:, 0:npc, :])

            # PE path: K rows [0, pe_rows_k), Q block 0 (rows 0-511)
            qk_piece_pe("k", k_dram, kt_all, DK, 0, 512)
            v_piece(0, 1024)
            qk_piece_pe("q", q_dram, qt_all, DQ, 0, 512)
            for r0 in range(512, pe_rows_k, 512):
                qk_piece_pe("k", k_dram, kt_all, DK, r0, 512)
            # DMA path, ordered by need time
            _, g1 = qk_piece_dma("k", k_dram, ksd, kt_all, DK,
                                 pe_rows_k, 2048 - pe_rows_k)
            v_piece(1024, 1024, after=g1)
            _, g2 = qk_piece_dma("q", q_dram, qsd, qt_all, DQ, 512, 1536, after=g1)
            _, g3 = qk_piece_dma("k", k_dram, ksd, kt_all, DK, 2048, 2048, after=g2)
            v_piece(2048, 2048, after=g2)
            _, g4 = qk_piece_dma("q", q_dram, qsd, qt_all, DQ, 2048, 2048, after=g3)

            # ---------------- main loop ----------------------------------
            gsizes = _groups(half_chunks, 3)   # (3,3,3,3,2,2)
            it = 0
            for _rep in range(iters):
              for h in range(2):
                for j in range(n_blocks):
                    ncol = slice(j * nb, (j + 1) * nb)
                    partial = partp.tile([P, ntiles, P], F32, tag="part", name="pt")
                    mc = h * half_chunks
                    n_g = len(gsizes)
                    for gi, g in enumerate(gsizes):
                        pool = sgpa if it % 2 == 0 else sgpb
                        s_g = pool.tile(
                            [P, g, nb], F32, tag=f"sg{it % 2}", name=f"sg{it % 2}"
                        )
                        it += 1
                        for ci in range(g):
                            cc = mc + ci
                            nc.tensor.matmul(
                                s_g[:, ci, :],
                                kt_all[0 : D + 1, cc * P : (cc + 1) * P],
                                qt_all[0 : D + 1, ncol],
                                start=True,
                                stop=True,
                            )
                        E = g * nb
                        a = int(E * act_frac) // 32 * 32
                        s_flat = s_g.rearrange("p g n -> p (g n)")
                        p_g = pexp.tile([P, g, nb], BF16, tag="pg", name="pg")
                        p_flat = p_g.rearrange("p g n -> p (g n)")
                        nc.scalar.activation(
                            out=p_flat[:, 0:a],
                            in_=s_flat[:, 0:a],
                            func=mybir.ActivationFunctionType.Exp,
                            scale=ACT_SCALE,
                            bias=bias_t,
                        )
                        nc.vector._custom_dve(
                            EXP2_OP,
                            out=p_flat[:, a:E].bitcast(I16),
                            in0=s_flat[:, a:E],
                            s0=K128,
                            s1=S_COEF,
                            imm2=C2_COEF,
                        )
                        for ci in range(g):
                            cc = mc + ci
                            for t in range(ntiles):
                                nc.tensor.matmul(
                                    partial[:, t, 0 : D + 1],
                                    p_g[:, ci, t * P : (t + 1) * P],
                                    v2[:, cc, :],
                                    start=(gi == 0 and ci == 0 and t == 0),
                                    stop=(gi == n_g - 1 and ci == g - 1 and t == ntiles - 1),
                                    skip_group_check=True,
                                )
                        mc += g

                    if h == 0:
                        nc.scalar.activation(
                            out=oaccs[j],
                            in_=partial[:, :, 0 : D + 1],
                            func=mybir.ActivationFunctionType.Copy,
                            scale=1.0,
                        )
                    else:
                        # partial -> SBUF on ACT; add + broadcast-mult on the
                        # (otherwise idle) Pool engine; DVE only does the
                        # small reciprocal.
                        ptmp = outp.tile([P, ntiles, D + 1], F32, tag="ptmp", name="ptmp")
                        nc.scalar.activation(
                            out=ptmp,
                            in_=partial[:, :, 0 : D + 1],
                            func=mybir.ActivationFunctionType.Copy,
                            scale=1.0,
                        )
                        osum = outp.tile([P, ntiles, D + 1], F32, tag="osum", name="osum")
                        nc.gpsimd.tensor_add(osum, oaccs[j], ptmp)
                        rinv = small.tile([P, ntiles, 1], F32, tag="rinv", name="rinv")
                        nc.vector.reciprocal(rinv, osum[:, :, D : D + 1])
                        o_sb = outp.tile([P, ntiles, D], F32, tag="osb", name="osb")
                        rinv_b = bass.AP(
                            tensor=rinv.tensor,
                            offset=rinv.offset,
                            ap=[rinv.ap[0], rinv.ap[1], [0, D]],
                        )
                        nc.gpsimd.tensor_tensor(
                            out=o_sb,
                            in0=osum[:, :, 0:D],
                            in1=rinv_b,
                            op=mybir.AluOpType.mult,
                        )
                        nc.sync.dma_start(
                            out=o_tiled[:, j * ntiles : (j + 1) * ntiles, :],
                            in_=o_sb,
                        )

    nc.compile()
    return nc


_NC_CACHE = {}


def _get_nc(**kw):
    key = tuple(sorted(kw.items()))
    if key not in _NC_CACHE:
        _NC_CACHE[key] = build_nc(**kw)
    return _NC_CACHE[key]


def kernel(queries, keys, values, **run_kwargs):
    """Full-input entry point: [8, 4096, 64] fp32 each -> [8, 4096, 64] fp32."""
    nc = _get_nc()
    in_maps = [
        {
            "queries": np.ascontiguousarray(queries[b], dtype=np.float32),
            "keys": np.ascontiguousarray(keys[b], dtype=np.float32),
            "values": np.ascontiguousarray(values[b], dtype=np.float32),
        }
        for b in range(B)
    ]
    res = run_bass_kernel_spmd(nc, in_maps, core_ids=list(range(B)), **run_kwargs)
    out = np.stack([res.results[b]["out"] for b in range(B)]).astype(np.float32)
    if run_kwargs:
        kernel.last_results = res
    return out
